# revision 32
# baseline (speedup 1.0000x reference)
"""Trainium2 Bass kernel for 7x7 sliding-window self-similarity attention.

out[b,c,h,w] = sum_j softmax_j(x[h,w] * x[h+dh,w+dw]) * x[h+dh,w+dw]
over the 7x7 neighborhood (zero padding, pad=3).

Sharding: B*C = 256 independent 128x128 images, 32 images per core on 8
NeuronCores (pure data parallel, no collectives).

Per-core: TWO batches of 16 images x 8 rowblocks of 16 rows; partition
p = rowblock(0..7)*16 + image(0..15); each partition holds a 28-row x
136-col zero-padded bf16 slab (3808 contiguous elems; 6-row / 4-col
halo), so every 7x7 shift is a flat offset view. Elementwise ops run on
contiguous 1D runs spanning the pad columns (finite garbage there,
never read).

Score symmetry: e_{-d}[i] == e_d[i-d]; only 25 canonical score tiles are
computed on an extended halo run; mirrored contributions are views.

Numerator trick: sum_d e_d[i]*x[i+d] = (sum of t_d = e_d*s_d views)/x[i]
(s_d is the score itself), so both the +d and -d numerator contributions
are views of one t tile; the final division by x cancels exactly:
out = acc_t / (x * sum_e).

Precision: everything bf16 except the PSUM accumulators and the final
division (validated l2 ~ 3e-3 vs the 2e-2 gate). bf16 makes DVE
tensor_tensor run in 2x packed mode and matmuls 1 cycle/row (fp32 is 4).

Engines: DVE does score and t=e*s products (bf16 2x) plus the final
division; ACT does exp (bf16 out); TensorE accumulates BOTH sum_e and
acc_t into PSUM via bf16 identity matmuls (the PE array is the
bottleneck at ~0.22 ns/row, so the PREADD view-pairs are pre-added on
DVE to shave PE work; 3 pre-adds balances PE ~163us vs DVE ~161us
busy). The two-batch split is what lets both accumulators (2 x 2048
fp32/partition) fit PSUM's 8 banks; PSUM lives as 4 half tiles so the
final division releases banks to the next batch early; the last
N_TAIL tiles emit e-matmuls before t-matmuls per chunk-pair so
den/recip/out hide under the trailing accumulation; and the first tile
is computed in three chunks so the PE starts as soon as the first
partial DMA lands. Output is bf16 (upcast on host) to halve the tail
DMA. ~185us measured unthrottled (baseline 615us).
"""

import numpy as np
import ml_dtypes

import concourse.bacc as bacc
import concourse.bass as bass  # noqa: F401
import concourse.tile as tile
from concourse import mybir
from concourse.bass_utils import run_bass_kernel_spmd

N_CORES = 8
F32 = mybir.dt.float32
BF16 = mybir.dt.bfloat16
NP_BF16 = ml_dtypes.bfloat16
MULT = mybir.AluOpType.mult
ADD = mybir.AluOpType.add

B, C, H, W = 4, 64, 128, 128
N_IMG_TOTAL = B * C
IMG_PER_CORE = N_IMG_TOTAL // N_CORES  # 32
N_BATCH = 2
IMG_PER_BATCH = IMG_PER_CORE // N_BATCH  # 16
RB_N = 8
BR = H // RB_N                 # 16 rows per rowblock
PADV = 6
PADH = 4
WP = W + 2 * PADH              # 136
SLAB = BR + 2 * PADV           # 28
NX = SLAB * WP                 # 3808 bf16 elems per partition per batch
P = IMG_PER_BATCH * RB_N       # 128 partitions
LA = BR * WP                   # 2176 full-width run
LC = BR * W                    # 2048 compact output per batch
T0 = PADV * WP + PADH          # 820: flat index of pixel (0,0)
LE = T0 + LA + 8               # tile size covering all runs
DF_MAX = 3 * WP + 3            # 411
MM_CHUNK = 512                 # one PSUM bank of fp32
N_CHUNKS = LC // MM_CHUNK      # 4
RPC = MM_CHUNK // W            # 4 rows per chunk
HALF = LC // 2                 # 1024 (two PSUM banks)

# canonical offsets, ordered: first tile warms the PE fast, the
# PREADD tiles sit mid-stream, (0,0) (single view) is last; the last
# N_TAIL tiles emit all their e-matmuls before their t-matmuls so the
# final division can overlap the trailing t accumulation
PREADD = [(1, 0), (2, 0), (0, 2)]
CANON = (
    [(3, 0)]
    + [(1, -3), (1, -2), (1, -1), (1, 0), (1, 1), (1, 2)]
    + [(1, 3), (2, -3), (2, -2), (2, 0), (2, -1), (2, 1)]
    + [(2, 2), (2, 3), (3, -3), (0, 2), (3, -2), (3, -1)]
    + [(3, 1), (3, 2), (3, 3)]
    + [(0, 1), (0, 3)]
    + [(0, 0)]
)
N_TAIL = 3                     # (0,1), (0,3), (0,0)
DF0 = 3 * WP                   # tile 0 is (3,0)


def view2d(ap, off, rows, cols, stride):
    """Strided [rows, cols] view at element offset `off` of a flat [P, L] AP."""
    a = ap.copy()
    pair_t = type(a.ap)
    part = list(a.ap)[0]
    a.ap = pair_t([list(part), [stride, rows], [1, cols]])
    a.offset = a.offset + off
    return a


def build_nc():
    nc = bacc.Bacc("TRN2", target_bir_lowering=False, debug=False)
    x_in = nc.dram_tensor("x", [P, N_BATCH * NX], BF16, kind="ExternalInput")
    id_in = nc.dram_tensor("ident", [P, P], BF16, kind="ExternalInput")
    y_out = nc.dram_tensor("y", [P, N_BATCH * LC], BF16,
                           kind="ExternalOutput")

    rd_lo = T0 - DF_MAX - 1            # 408
    rd_hi = T0 + LA + DF_MAX + 1       # 3408
    lo0 = T0 - DF0                     # 412
    ln0 = LA + DF0
    # tile 0 computed in three chunks so the PE starts on the first
    # partial DMA; chunk boundaries even-aligned
    cut1 = lo0 + 648                   # 1060
    cut2 = lo0 + ln0 // 2 + 2          # 1704
    cuts0 = [lo0, cut1, cut2, lo0 + ln0]

    with tile.TileContext(nc) as tc:
        with (
            tc.tile_pool(name="big", bufs=1) as big,
            tc.tile_pool(name="sp", bufs=4) as spool,
            tc.tile_pool(name="ep", bufs=4) as epool,
            tc.tile_pool(name="tp", bufs=4) as tpool,
            tc.tile_pool(name="up", bufs=3) as upool,
            tc.tile_pool(name="fin", bufs=2) as fin,
            tc.tile_pool(name="ps", bufs=1, space="PSUM") as ps,
        ):
            x = big.tile([P, N_BATCH * NX], BF16, tag="x")
            ident = big.tile([P, P], BF16, tag="id")

            # batch 0 input in 3 pieces matching tile 0's chunk needs
            d1 = cut1 + DF0
            d2 = cut2 + DF0
            # d2/d3 go out on the Activation engine's DMA queue so their
            # transfers overlap d1's on the sync queue
            nc.sync.dma_start(out=x[:, rd_lo:d1], in_=x_in[:, rd_lo:d1])
            nc.scalar.dma_start(out=x[:, d1:d2], in_=x_in[:, d1:d2])
            nc.scalar.dma_start(out=x[:, d2:rd_hi], in_=x_in[:, d2:rd_hi])
            nc.sync.dma_start(out=ident[:], in_=id_in[:])
            # batch 1 range streams under batch 0 compute
            nc.sync.dma_start(out=x[:, NX + rd_lo:NX + rd_hi],
                              in_=x_in[:, NX + rd_lo:NX + rd_hi])

            def emit_tile(b, k):
                base = b * NX
                di, dj = CANON[k]
                df = di * WP + dj
                lo = T0 - df
                ln = LA + df
                al = lo & 1
                lo -= al
                ln += al
                s = spool.tile([P, LE], BF16, tag="s")
                e = epool.tile([P, LE], BF16, tag="e")
                t = tpool.tile([P, LE], BF16, tag="t")
                # split the very first tile so the PE starts early
                if b == 0 and k == 0:
                    cuts = cuts0
                else:
                    cuts = [lo, lo + ln]
                for ci in range(len(cuts) - 1):
                    c0, c1 = cuts[ci], cuts[ci + 1]
                    sv = s[:, c0:c1]
                    ev = e[:, c0:c1]
                    tv = t[:, c0:c1]
                    if df == 0:
                        nc.scalar.activation(
                            out=sv, in_=x[:, base + c0:base + c1],
                            func=mybir.ActivationFunctionType.Square,
                        )
                    else:
                        nc.vector.tensor_tensor(
                            out=sv,
                            in0=x[:, base + c0:base + c1],
                            in1=x[:, base + c0 + df:base + c1 + df],
                            op=MULT,
                        )
                    nc.scalar.activation(
                        out=ev, in_=sv, func=mybir.ActivationFunctionType.Exp
                    )
                    nc.vector.tensor_tensor(out=tv, in0=ev, in1=sv, op=MULT)
                return s, e, t, df

            def emit_preadd(tiles):
                s, e, t, df = tiles
                # collapse the two views into one via a DVE pre-add
                ue = upool.tile([P, LA], BF16, tag="ue")
                ut = upool.tile([P, LA], BF16, tag="ut")
                nc.vector.tensor_tensor(
                    out=ue[:], in0=e[:, T0:T0 + LA],
                    in1=e[:, T0 - df:T0 - df + LA], op=ADD)
                nc.vector.tensor_tensor(
                    out=ut[:], in0=t[:, T0:T0 + LA],
                    in1=t[:, T0 - df:T0 - df + LA], op=ADD)
                return ue, ut

            def emit_mms(k, tiles, pe, pt, first, chunks=range(N_CHUNKS),
                         qs="et", pre=None):
                s, e, t, df = tiles
                stop = k == len(CANON) - 1
                if pre is not None:
                    ue, ut = pre
                    views = [(ue, ut, 0)]
                else:
                    views = [(e, t, T0)]
                    if df != 0:
                        views.append((e, t, T0 - df))
                    if k == 0:
                        # the T0-df view's first chunks only need the first
                        # input cut, so emit that view first at kernel start
                        views.reverse()
                for esrc, tsrc, to in views:
                    for ci in chunks:
                        if "e" in qs:
                            pev = pe[ci // 2][:, (ci % 2) * MM_CHUNK:
                                              (ci % 2 + 1) * MM_CHUNK]
                            mve = view2d(esrc[:], to + ci * RPC * WP,
                                         RPC, W, WP)
                            nc.tensor.matmul(pev, ident[:], mve,
                                             start=first[ci], stop=stop)
                        if "t" in qs:
                            ptv = pt[ci // 2][:, (ci % 2) * MM_CHUNK:
                                              (ci % 2 + 1) * MM_CHUNK]
                            mvt = view2d(tsrc[:], to + ci * RPC * WP,
                                         RPC, W, WP)
                            nc.tensor.matmul(ptv, ident[:], mvt,
                                             start=first[ci], stop=stop)
                        if qs == "et":
                            first[ci] = False

            def emit_one(k, tiles, pe, pt, first, chunks=range(N_CHUNKS)):
                pre = emit_preadd(tiles) if CANON[k] in PREADD else None
                emit_mms(k, tiles, pe, pt, first, chunks, pre=pre)

            def emit_final_dr(b, h, pe):
                base = b * NX
                xc = view2d(x[:], base + T0 + h * (BR // 2) * WP,
                            BR // 2, W, WP)
                den = fin.tile([P, HALF], F32, tag="den%d" % h)
                r = fin.tile([P, HALF], F32, tag="r%d" % h)
                nc.vector.tensor_tensor(out=den[:], in0=pe[h][:], in1=xc,
                                        op=MULT)
                nc.vector.reciprocal_approx_fast(out=r[:], in_=den[:])
                return r

            def emit_final_out(b, h, pt, r):
                out = fin.tile([P, HALF], BF16, tag="out%d" % h)
                nc.vector.tensor_tensor(out=out[:], in0=pt[h][:], in1=r[:],
                                        op=MULT)
                nc.sync.dma_start(
                    out=y_out[:, b * LC + h * HALF:b * LC + (h + 1) * HALF],
                    in_=out[:])

            def emit_batch_tail(b, pe, pt, first, last=False):
                # all e-matmuls of the last N_TAIL tiles before their
                # t-matmuls, in chunk-pair order: closes each PSUM half
                # early so den/recip/out run under the trailing matmuls
                ks = list(range(len(CANON) - N_TAIL, len(CANON)))
                tls = [emit_tile(b, k) for k in ks]
                rs = [None, None]
                for cp in ((0, 1), (2, 3)):
                    for k, tl in zip(ks, tls):
                        emit_mms(k, tl, pe, pt, first, chunks=cp, qs="e")
                    rs[cp[0] // 2] = emit_final_dr(b, cp[0] // 2, pe)
                for cp in ((0, 1), (2, 3)):
                    for k, tl in zip(ks, tls):
                        emit_mms(k, tl, pe, pt, first, chunks=cp, qs="t")
                    if last:
                        emit_final_out(b, cp[0] // 2, pt, rs[cp[0] // 2])
                return rs

            pt_prev = rs_prev = None
            n_mid = len(CANON) - N_TAIL
            for b in range(N_BATCH):
                pe = [ps.tile([P, HALF], F32, tag="pe%d" % i,
                              name="pe%d" % i) for i in (0, 1)]
                pt = [ps.tile([P, HALF], F32, tag="pt%d" % i,
                              name="pt%d" % i) for i in (0, 1)]
                first = [True] * N_CHUNKS
                if b == 0:
                    for k in range(n_mid):
                        emit_one(k, emit_tile(b, k), pe, pt, first)
                else:
                    # den/recip for the previous batch already ran under its
                    # tail; only out+DMA (which release the pt banks) remain,
                    # interleaved with this batch's first tile computes
                    t0_tiles = emit_tile(b, 0)
                    emit_final_out(b - 1, 0, pt_prev, rs_prev[0])
                    emit_one(0, t0_tiles, pe, pt, first, chunks=(0, 1))
                    t1_tiles = emit_tile(b, 1)
                    emit_final_out(b - 1, 1, pt_prev, rs_prev[1])
                    emit_one(0, t0_tiles, pe, pt, first, chunks=(2, 3))
                    emit_one(1, t1_tiles, pe, pt, first)
                    for k in range(2, n_mid):
                        emit_one(k, emit_tile(b, k), pe, pt, first)
                rs_prev = emit_batch_tail(b, pe, pt, first,
                                          last=(b == N_BATCH - 1))
                pt_prev = pt
    nc.compile()
    return nc


_NC_CACHE = {}


def _get_nc():
    if "nc" not in _NC_CACHE:
        _NC_CACHE["nc"] = build_nc()
    return _NC_CACHE["nc"]


def make_slabs(imgs):
    """[32,128,128] fp32 (one core) -> [128, 2*3808] bf16 slab layout."""
    xb = imgs.astype(NP_BF16)
    xp = np.pad(xb, ((0, 0), (PADV, PADV), (PADH, PADH)))
    rows = (np.arange(RB_N) * BR)[:, None] + np.arange(SLAB)
    out = np.empty((P, N_BATCH, NX), NP_BF16)
    for b in range(N_BATCH):
        part = xp[b * IMG_PER_BATCH:(b + 1) * IMG_PER_BATCH]  # [16,140,136]
        sl = part[:, rows, :]              # [16, 8, 28, 136]
        sl = sl.transpose(1, 0, 2, 3)      # [8, 16, 28, 136] p = rb*16+img
        out[:, b, :] = sl.reshape(P, NX)
    return np.ascontiguousarray(out.reshape(P, N_BATCH * NX))


def unslab_out(y):
    """[128, 2*2048] bf16 -> [32, 128, 128] fp32."""
    y = y.astype(np.float32)
    res = np.empty((IMG_PER_CORE, H, W), np.float32)
    for b in range(N_BATCH):
        yb = y[:, b * LC:(b + 1) * LC].reshape(RB_N, IMG_PER_BATCH, BR, W)
        res[b * IMG_PER_BATCH:(b + 1) * IMG_PER_BATCH] = (
            yb.transpose(1, 0, 2, 3).reshape(IMG_PER_BATCH, H, W)
        )
    return res


def run(x, **spmd_kwargs):
    nc = _get_nc()
    imgs = np.ascontiguousarray(np.asarray(x).reshape(N_IMG_TOTAL, H, W))
    imgs = imgs.astype(np.float32, copy=False)
    ident = np.eye(P, dtype=NP_BF16)
    in_maps = [
        {"x": make_slabs(imgs[i * IMG_PER_CORE:(i + 1) * IMG_PER_CORE]),
         "ident": ident}
        for i in range(N_CORES)
    ]
    res = run_bass_kernel_spmd(nc, in_maps, core_ids=list(range(N_CORES)),
                               **spmd_kwargs)
    out = np.concatenate(
        [unslab_out(res.results[i]["y"]) for i in range(N_CORES)],
        axis=0,
    )
    return out.reshape(B, C, H, W).astype(np.float32, copy=False), res


def kernel(x):
    out, _ = run(x)
    return out


# revision 33
# speedup vs baseline: 1.2041x; 1.2041x over previous
"""Trainium2 Bass kernel for 7x7 sliding-window self-similarity attention.

out[b,c,h,w] = sum_j softmax_j(x[h,w] * x[h+dh,w+dw]) * x[h+dh,w+dw]
over the 7x7 neighborhood (zero padding, pad=3).

Sharding: B*C = 256 independent 128x128 images, 32 images per core on 8
NeuronCores (pure data parallel, no collectives).

Per-core: TWO batches of 16 images x 8 rowblocks of 16 rows; partition
p = rowblock(0..7)*16 + image(0..15); each partition holds a 28-row x
136-col zero-padded bf16 slab (3808 contiguous elems; 6-row / 4-col
halo), so every 7x7 shift is a flat offset view. Elementwise ops run on
contiguous 1D runs spanning the pad columns (finite garbage there,
never read).

Score symmetry: e_{-d}[i] == e_d[i-d]; only 25 canonical score tiles are
computed on an extended halo run; mirrored contributions are views.

Numerator trick: sum_d e_d[i]*x[i+d] = (sum of t_d = e_d*s_d views)/x[i]
(s_d is the score itself), so both the +d and -d numerator contributions
are views of one t tile; the final division by x cancels exactly:
out = acc_t / (x * sum_e).

Precision: everything bf16 except the PSUM accumulators and the final
division (validated l2 ~ 3e-3 vs the 2e-2 gate). bf16 makes DVE
tensor_tensor run in 2x packed mode and matmuls 1 cycle/row (fp32 is 4).

Engines: DVE does score and t=e*s products (bf16 2x) plus the final
division; ACT does exp (bf16 out); TensorE accumulates BOTH sum_e and
acc_t into PSUM via bf16 identity matmuls (the PE array is the
bottleneck at ~0.22 ns/row, so the PREADD view-pairs are pre-added on
DVE to shave PE work; 3 pre-adds balances PE ~163us vs DVE ~161us
busy). The two-batch split is what lets both accumulators (2 x 2048
fp32/partition) fit PSUM's 8 banks; PSUM lives as 4 half tiles so the
final division releases banks to the next batch early; the last
N_TAIL tiles emit e-matmuls before t-matmuls per chunk-pair so
den/recip/out hide under the trailing accumulation; and the first tile
is computed in three chunks so the PE starts as soon as the first
partial DMA lands. Output is bf16 (upcast on host) to halve the tail
DMA. ~185us measured unthrottled (baseline 615us).
"""

import numpy as np
import ml_dtypes

import concourse.bacc as bacc
import concourse.bass as bass  # noqa: F401
import concourse.tile as tile
from concourse import mybir
from concourse.bass_utils import run_bass_kernel_spmd

N_CORES = 8
F32 = mybir.dt.float32
BF16 = mybir.dt.bfloat16
NP_BF16 = ml_dtypes.bfloat16
MULT = mybir.AluOpType.mult
ADD = mybir.AluOpType.add

B, C, H, W = 4, 64, 128, 128
N_IMG_TOTAL = B * C
IMG_PER_CORE = N_IMG_TOTAL // N_CORES  # 32
N_BATCH = 2
IMG_PER_BATCH = IMG_PER_CORE // N_BATCH  # 16
RB_N = 8
BR = H // RB_N                 # 16 rows per rowblock
PADV = 6
PADH = 4
WP = W + 2 * PADH              # 136
SLAB = BR + 2 * PADV           # 28
NX = SLAB * WP                 # 3808 bf16 elems per partition per batch
P = IMG_PER_BATCH * RB_N       # 128 partitions
LA = BR * WP                   # 2176 full-width run
LC = BR * W                    # 2048 compact output per batch
T0 = PADV * WP + PADH          # 820: flat index of pixel (0,0)
LE = T0 + LA + 8               # tile size covering all runs
DF_MAX = 3 * WP + 3            # 411
MM_CHUNK = 512                 # one PSUM bank of fp32
N_CHUNKS = LC // MM_CHUNK      # 4
RPC = MM_CHUNK // W            # 4 rows per chunk
HALF = LC // 2                 # 1024 (two PSUM banks)

# canonical offsets, ordered: first tile warms the PE fast, the
# PREADD tiles sit mid-stream, (0,0) (single view) is last; the last
# N_TAIL tiles emit all their e-matmuls before their t-matmuls so the
# final division can overlap the trailing t accumulation
PREADD = [(1, 0), (2, 0), (0, 2)]
CANON = (
    [(3, 0)]
    + [(1, -3), (1, -2), (1, -1), (1, 0), (1, 1), (1, 2)]
    + [(1, 3), (2, -3), (2, -2), (2, 0), (2, -1), (2, 1)]
    + [(2, 2), (2, 3), (3, -3), (0, 2), (3, -2), (3, -1)]
    + [(3, 1), (3, 2), (3, 3)]
    + [(0, 1), (0, 3)]
    + [(0, 0)]
)
N_TAIL = 3                     # (0,1), (0,3), (0,0)
DF0 = 3 * WP                   # tile 0 is (3,0)


def view2d(ap, off, rows, cols, stride):
    """Strided [rows, cols] view at element offset `off` of a flat [P, L] AP."""
    a = ap.copy()
    pair_t = type(a.ap)
    part = list(a.ap)[0]
    a.ap = pair_t([list(part), [stride, rows], [1, cols]])
    a.offset = a.offset + off
    return a


def build_nc():
    nc = bacc.Bacc("TRN2", target_bir_lowering=False, debug=False)
    x_in = nc.dram_tensor("x", [P, N_BATCH * NX], BF16, kind="ExternalInput")
    id_in = nc.dram_tensor("ident", [P, P], BF16, kind="ExternalInput")
    y_out = nc.dram_tensor("y", [P, N_BATCH * LC], BF16,
                           kind="ExternalOutput")

    rd_lo = T0 - DF_MAX - 1            # 408
    rd_hi = T0 + LA + DF_MAX + 1       # 3408
    lo0 = T0 - DF0                     # 412
    ln0 = LA + DF0
    # tile 0 computed in three chunks so the PE starts on the first
    # partial DMA; chunk boundaries even-aligned
    cut1 = lo0 + 648                   # 1060
    cut2 = lo0 + ln0 // 2 + 2          # 1704
    cuts0 = [lo0, cut1, cut2, lo0 + ln0]

    with tile.TileContext(nc) as tc:
        with (
            tc.tile_pool(name="big", bufs=1) as big,
            tc.tile_pool(name="sp", bufs=4) as spool,
            tc.tile_pool(name="ep", bufs=4) as epool,
            tc.tile_pool(name="tp", bufs=4) as tpool,
            tc.tile_pool(name="up", bufs=3) as upool,
            tc.tile_pool(name="fin", bufs=2) as fin,
            tc.tile_pool(name="ps", bufs=1, space="PSUM") as ps,
        ):
            x = big.tile([P, N_BATCH * NX], BF16, tag="x")
            ident = big.tile([P, P], BF16, tag="id")

            # batch 0 input in 3 pieces matching tile 0's chunk needs
            d1 = cut1 + DF0
            d2 = cut2 + DF0
            nc.sync.dma_start(out=x[:, rd_lo:d1], in_=x_in[:, rd_lo:d1])
            nc.sync.dma_start(out=x[:, d1:d2], in_=x_in[:, d1:d2])
            nc.sync.dma_start(out=x[:, d2:rd_hi], in_=x_in[:, d2:rd_hi])
            nc.sync.dma_start(out=ident[:], in_=id_in[:])
            # batch 1 range streams under batch 0 compute
            nc.sync.dma_start(out=x[:, NX + rd_lo:NX + rd_hi],
                              in_=x_in[:, NX + rd_lo:NX + rd_hi])

            def emit_tile(b, k):
                base = b * NX
                di, dj = CANON[k]
                df = di * WP + dj
                lo = T0 - df
                ln = LA + df
                al = lo & 1
                lo -= al
                ln += al
                s = spool.tile([P, LE], BF16, tag="s")
                e = epool.tile([P, LE], BF16, tag="e")
                t = tpool.tile([P, LE], BF16, tag="t")
                # split the very first tile so the PE starts early
                if b == 0 and k == 0:
                    cuts = cuts0
                else:
                    cuts = [lo, lo + ln]
                for ci in range(len(cuts) - 1):
                    c0, c1 = cuts[ci], cuts[ci + 1]
                    sv = s[:, c0:c1]
                    ev = e[:, c0:c1]
                    tv = t[:, c0:c1]
                    if df == 0:
                        nc.scalar.activation(
                            out=sv, in_=x[:, base + c0:base + c1],
                            func=mybir.ActivationFunctionType.Square,
                        )
                    else:
                        nc.vector.tensor_tensor(
                            out=sv,
                            in0=x[:, base + c0:base + c1],
                            in1=x[:, base + c0 + df:base + c1 + df],
                            op=MULT,
                        )
                    nc.scalar.activation(
                        out=ev, in_=sv, func=mybir.ActivationFunctionType.Exp
                    )
                    nc.vector.tensor_tensor(out=tv, in0=ev, in1=sv, op=MULT)
                return s, e, t, df

            def emit_preadd(tiles):
                s, e, t, df = tiles
                # collapse the two views into one via a DVE pre-add
                ue = upool.tile([P, LA], BF16, tag="ue")
                ut = upool.tile([P, LA], BF16, tag="ut")
                nc.vector.tensor_tensor(
                    out=ue[:], in0=e[:, T0:T0 + LA],
                    in1=e[:, T0 - df:T0 - df + LA], op=ADD)
                nc.vector.tensor_tensor(
                    out=ut[:], in0=t[:, T0:T0 + LA],
                    in1=t[:, T0 - df:T0 - df + LA], op=ADD)
                return ue, ut

            def emit_mms(k, tiles, pe, pt, first, chunks=range(N_CHUNKS),
                         qs="et", pre=None):
                s, e, t, df = tiles
                stop = k == len(CANON) - 1
                if pre is not None:
                    ue, ut = pre
                    views = [(ue, ut, 0)]
                else:
                    views = [(e, t, T0)]
                    if df != 0:
                        views.append((e, t, T0 - df))
                    if k == 0:
                        # the T0-df view's first chunks only need the first
                        # input cut, so emit that view first at kernel start
                        views.reverse()
                for esrc, tsrc, to in views:
                    for ci in chunks:
                        if "e" in qs:
                            pev = pe[ci // 2][:, (ci % 2) * MM_CHUNK:
                                              (ci % 2 + 1) * MM_CHUNK]
                            mve = view2d(esrc[:], to + ci * RPC * WP,
                                         RPC, W, WP)
                            nc.tensor.matmul(pev, ident[:], mve,
                                             start=first[ci], stop=stop)
                        if "t" in qs:
                            ptv = pt[ci // 2][:, (ci % 2) * MM_CHUNK:
                                              (ci % 2 + 1) * MM_CHUNK]
                            mvt = view2d(tsrc[:], to + ci * RPC * WP,
                                         RPC, W, WP)
                            nc.tensor.matmul(ptv, ident[:], mvt,
                                             start=first[ci], stop=stop)
                        if qs == "et":
                            first[ci] = False

            def emit_one(k, tiles, pe, pt, first, chunks=range(N_CHUNKS)):
                pre = emit_preadd(tiles) if CANON[k] in PREADD else None
                emit_mms(k, tiles, pe, pt, first, chunks, pre=pre)

            def emit_final_dr(b, h, pe):
                base = b * NX
                xc = view2d(x[:], base + T0 + h * (BR // 2) * WP,
                            BR // 2, W, WP)
                den = fin.tile([P, HALF], F32, tag="den%d" % h)
                r = fin.tile([P, HALF], F32, tag="r%d" % h)
                nc.vector.tensor_tensor(out=den[:], in0=pe[h][:], in1=xc,
                                        op=MULT)
                nc.vector.reciprocal_approx_fast(out=r[:], in_=den[:])
                return r

            def emit_final_out(b, h, pt, r):
                out = fin.tile([P, HALF], BF16, tag="out%d" % h)
                nc.vector.tensor_tensor(out=out[:], in0=pt[h][:], in1=r[:],
                                        op=MULT)
                nc.sync.dma_start(
                    out=y_out[:, b * LC + h * HALF:b * LC + (h + 1) * HALF],
                    in_=out[:])

            def emit_batch_tail(b, pe, pt, first, last=False):
                # all e-matmuls of the last N_TAIL tiles before their
                # t-matmuls, in chunk-pair order: closes each PSUM half
                # early so den/recip/out run under the trailing matmuls
                ks = list(range(len(CANON) - N_TAIL, len(CANON)))
                tls = [emit_tile(b, k) for k in ks]
                rs = [None, None]
                for cp in ((0, 1), (2, 3)):
                    for k, tl in zip(ks, tls):
                        emit_mms(k, tl, pe, pt, first, chunks=cp, qs="e")
                    rs[cp[0] // 2] = emit_final_dr(b, cp[0] // 2, pe)
                for cp in ((0, 1), (2, 3)):
                    for k, tl in zip(ks, tls):
                        emit_mms(k, tl, pe, pt, first, chunks=cp, qs="t")
                    if last:
                        emit_final_out(b, cp[0] // 2, pt, rs[cp[0] // 2])
                return rs

            pt_prev = rs_prev = None
            n_mid = len(CANON) - N_TAIL
            for b in range(N_BATCH):
                pe = [ps.tile([P, HALF], F32, tag="pe%d" % i,
                              name="pe%d" % i) for i in (0, 1)]
                pt = [ps.tile([P, HALF], F32, tag="pt%d" % i,
                              name="pt%d" % i) for i in (0, 1)]
                first = [True] * N_CHUNKS
                if b == 0:
                    for k in range(n_mid):
                        emit_one(k, emit_tile(b, k), pe, pt, first)
                else:
                    # den/recip for the previous batch already ran under its
                    # tail; only out+DMA (which release the pt banks) remain,
                    # interleaved with this batch's first tile computes
                    t0_tiles = emit_tile(b, 0)
                    emit_final_out(b - 1, 0, pt_prev, rs_prev[0])
                    emit_one(0, t0_tiles, pe, pt, first, chunks=(0, 1))
                    t1_tiles = emit_tile(b, 1)
                    emit_final_out(b - 1, 1, pt_prev, rs_prev[1])
                    emit_one(0, t0_tiles, pe, pt, first, chunks=(2, 3))
                    emit_one(1, t1_tiles, pe, pt, first)
                    for k in range(2, n_mid):
                        emit_one(k, emit_tile(b, k), pe, pt, first)
                rs_prev = emit_batch_tail(b, pe, pt, first,
                                          last=(b == N_BATCH - 1))
                pt_prev = pt
    nc.compile()
    return nc


_NC_CACHE = {}


def _get_nc():
    if "nc" not in _NC_CACHE:
        _NC_CACHE["nc"] = build_nc()
    return _NC_CACHE["nc"]


def make_slabs(imgs):
    """[32,128,128] fp32 (one core) -> [128, 2*3808] bf16 slab layout."""
    xb = imgs.astype(NP_BF16)
    xp = np.pad(xb, ((0, 0), (PADV, PADV), (PADH, PADH)))
    rows = (np.arange(RB_N) * BR)[:, None] + np.arange(SLAB)
    out = np.empty((P, N_BATCH, NX), NP_BF16)
    for b in range(N_BATCH):
        part = xp[b * IMG_PER_BATCH:(b + 1) * IMG_PER_BATCH]  # [16,140,136]
        sl = part[:, rows, :]              # [16, 8, 28, 136]
        sl = sl.transpose(1, 0, 2, 3)      # [8, 16, 28, 136] p = rb*16+img
        out[:, b, :] = sl.reshape(P, NX)
    return np.ascontiguousarray(out.reshape(P, N_BATCH * NX))


def unslab_out(y):
    """[128, 2*2048] bf16 -> [32, 128, 128] fp32."""
    y = y.astype(np.float32)
    res = np.empty((IMG_PER_CORE, H, W), np.float32)
    for b in range(N_BATCH):
        yb = y[:, b * LC:(b + 1) * LC].reshape(RB_N, IMG_PER_BATCH, BR, W)
        res[b * IMG_PER_BATCH:(b + 1) * IMG_PER_BATCH] = (
            yb.transpose(1, 0, 2, 3).reshape(IMG_PER_BATCH, H, W)
        )
    return res


def run(x, **spmd_kwargs):
    nc = _get_nc()
    imgs = np.ascontiguousarray(np.asarray(x).reshape(N_IMG_TOTAL, H, W))
    imgs = imgs.astype(np.float32, copy=False)
    ident = np.eye(P, dtype=NP_BF16)
    in_maps = [
        {"x": make_slabs(imgs[i * IMG_PER_CORE:(i + 1) * IMG_PER_CORE]),
         "ident": ident}
        for i in range(N_CORES)
    ]
    res = run_bass_kernel_spmd(nc, in_maps, core_ids=list(range(N_CORES)),
                               **spmd_kwargs)
    out = np.concatenate(
        [unslab_out(res.results[i]["y"]) for i in range(N_CORES)],
        axis=0,
    )
    return out.reshape(B, C, H, W).astype(np.float32, copy=False), res


def kernel(x):
    out, _ = run(x)
    return out


# revision 34
# speedup vs baseline: 1.2061x; 1.0017x over previous
"""Trainium2 Bass kernel for 7x7 sliding-window self-similarity attention.

out[b,c,h,w] = sum_j softmax_j(x[h,w] * x[h+dh,w+dw]) * x[h+dh,w+dw]
over the 7x7 neighborhood (zero padding, pad=3).

Sharding: B*C = 256 independent 128x128 images, 32 images per core on 8
NeuronCores (pure data parallel, no collectives).

Per-core: TWO batches of 16 images x 8 rowblocks of 16 rows; partition
p = rowblock(0..7)*16 + image(0..15); each partition holds a 28-row x
136-col zero-padded bf16 slab (3808 contiguous elems; 6-row / 4-col
halo), so every 7x7 shift is a flat offset view. Elementwise ops run on
contiguous 1D runs spanning the pad columns (finite garbage there,
never read).

Score symmetry: e_{-d}[i] == e_d[i-d]; only 25 canonical score tiles are
computed on an extended halo run; mirrored contributions are views.

Numerator trick: sum_d e_d[i]*x[i+d] = (sum of t_d = e_d*s_d views)/x[i]
(s_d is the score itself), so both the +d and -d numerator contributions
are views of one t tile; the final division by x cancels exactly:
out = acc_t / (x * sum_e).

Precision: everything bf16 except the PSUM accumulators and the final
division (validated l2 ~ 3e-3 vs the 2e-2 gate). bf16 makes DVE
tensor_tensor run in 2x packed mode and matmuls 1 cycle/row (fp32 is 4).

Engines: DVE does score and t=e*s products (bf16 2x) plus the final
division; ACT does exp (bf16 out); TensorE accumulates BOTH sum_e and
acc_t into PSUM via bf16 identity matmuls (the PE array is the
bottleneck at ~0.22 ns/row, so the PREADD view-pairs are pre-added on
DVE to shave PE work; 3 pre-adds balances PE ~163us vs DVE ~161us
busy). The two-batch split is what lets both accumulators (2 x 2048
fp32/partition) fit PSUM's 8 banks; PSUM lives as 4 half tiles so the
final division releases banks to the next batch early; the last
N_TAIL tiles emit e-matmuls before t-matmuls per chunk-pair so
den/recip/out hide under the trailing accumulation; and the first tile
is computed in three chunks so the PE starts as soon as the first
partial DMA lands. Output is bf16 (upcast on host) to halve the tail
DMA. ~185us measured unthrottled (baseline 615us).
"""

import numpy as np
import ml_dtypes

import concourse.bacc as bacc
import concourse.bass as bass  # noqa: F401
import concourse.tile as tile
from concourse import mybir
from concourse.bass_utils import run_bass_kernel_spmd

N_CORES = 8
F32 = mybir.dt.float32
BF16 = mybir.dt.bfloat16
NP_BF16 = ml_dtypes.bfloat16
MULT = mybir.AluOpType.mult
ADD = mybir.AluOpType.add

B, C, H, W = 4, 64, 128, 128
N_IMG_TOTAL = B * C
IMG_PER_CORE = N_IMG_TOTAL // N_CORES  # 32
N_BATCH = 2
IMG_PER_BATCH = IMG_PER_CORE // N_BATCH  # 16
RB_N = 8
BR = H // RB_N                 # 16 rows per rowblock
PADV = 6
PADH = 4
WP = W + 2 * PADH              # 136
SLAB = BR + 2 * PADV           # 28
NX = SLAB * WP                 # 3808 bf16 elems per partition per batch
P = IMG_PER_BATCH * RB_N       # 128 partitions
LA = BR * WP                   # 2176 full-width run
LC = BR * W                    # 2048 compact output per batch
T0 = PADV * WP + PADH          # 820: flat index of pixel (0,0)
LE = T0 + LA + 8               # tile size covering all runs
DF_MAX = 3 * WP + 3            # 411
MM_CHUNK = 512                 # one PSUM bank of fp32
N_CHUNKS = LC // MM_CHUNK      # 4
RPC = MM_CHUNK // W            # 4 rows per chunk
HALF = LC // 2                 # 1024 (two PSUM banks)

# canonical offsets, ordered: first tile warms the PE fast, the
# PREADD tiles sit mid-stream, (0,0) (single view) is last; the last
# N_TAIL tiles emit all their e-matmuls before their t-matmuls so the
# final division can overlap the trailing t accumulation
PREADD = [(1, 0), (2, 0), (0, 2)]
CANON = (
    [(3, 0)]
    + [(1, -3), (1, -2), (1, -1), (1, 0), (1, 1), (1, 2)]
    + [(1, 3), (2, -3), (2, -2), (2, 0), (2, -1), (2, 1)]
    + [(2, 2), (2, 3), (3, -3), (0, 2), (3, -2), (3, -1)]
    + [(3, 1), (3, 2), (3, 3)]
    + [(0, 1), (0, 3)]
    + [(0, 0)]
)
N_TAIL = 4                     # (3,3), (0,1), (0,3), (0,0)
DF0 = 3 * WP                   # tile 0 is (3,0)


def view2d(ap, off, rows, cols, stride):
    """Strided [rows, cols] view at element offset `off` of a flat [P, L] AP."""
    a = ap.copy()
    pair_t = type(a.ap)
    part = list(a.ap)[0]
    a.ap = pair_t([list(part), [stride, rows], [1, cols]])
    a.offset = a.offset + off
    return a


def build_nc():
    nc = bacc.Bacc("TRN2", target_bir_lowering=False, debug=False)
    x_in = nc.dram_tensor("x", [P, N_BATCH * NX], BF16, kind="ExternalInput")
    id_in = nc.dram_tensor("ident", [P, P], BF16, kind="ExternalInput")
    y_out = nc.dram_tensor("y", [P, N_BATCH * LC], BF16,
                           kind="ExternalOutput")

    rd_lo = T0 - DF_MAX - 1            # 408
    rd_hi = T0 + LA + DF_MAX + 1       # 3408
    lo0 = T0 - DF0                     # 412
    ln0 = LA + DF0
    # tile 0 computed in three chunks so the PE starts on the first
    # partial DMA; chunk boundaries even-aligned
    cut1 = lo0 + 648                   # 1060
    cut2 = lo0 + ln0 // 2 + 2          # 1704
    cuts0 = [lo0, cut1, cut2, lo0 + ln0]

    with tile.TileContext(nc) as tc:
        with (
            tc.tile_pool(name="big", bufs=1) as big,
            tc.tile_pool(name="sp", bufs=4) as spool,
            tc.tile_pool(name="ep", bufs=4) as epool,
            tc.tile_pool(name="tp", bufs=4) as tpool,
            tc.tile_pool(name="up", bufs=3) as upool,
            tc.tile_pool(name="fin", bufs=2) as fin,
            tc.tile_pool(name="ps", bufs=1, space="PSUM") as ps,
        ):
            x = big.tile([P, N_BATCH * NX], BF16, tag="x")
            ident = big.tile([P, P], BF16, tag="id")

            # batch 0 input in 3 pieces matching tile 0's chunk needs
            d1 = cut1 + DF0
            d2 = cut2 + DF0
            nc.sync.dma_start(out=x[:, rd_lo:d1], in_=x_in[:, rd_lo:d1])
            nc.sync.dma_start(out=x[:, d1:d2], in_=x_in[:, d1:d2])
            nc.sync.dma_start(out=x[:, d2:rd_hi], in_=x_in[:, d2:rd_hi])
            nc.sync.dma_start(out=ident[:], in_=id_in[:])
            # batch 1 range streams under batch 0 compute
            nc.sync.dma_start(out=x[:, NX + rd_lo:NX + rd_hi],
                              in_=x_in[:, NX + rd_lo:NX + rd_hi])

            def emit_tile(b, k):
                base = b * NX
                di, dj = CANON[k]
                df = di * WP + dj
                lo = T0 - df
                ln = LA + df
                al = lo & 1
                lo -= al
                ln += al
                s = spool.tile([P, LE], BF16, tag="s")
                e = epool.tile([P, LE], BF16, tag="e")
                t = tpool.tile([P, LE], BF16, tag="t")
                # split the very first tile so the PE starts early
                if b == 0 and k == 0:
                    cuts = cuts0
                else:
                    cuts = [lo, lo + ln]
                for ci in range(len(cuts) - 1):
                    c0, c1 = cuts[ci], cuts[ci + 1]
                    sv = s[:, c0:c1]
                    ev = e[:, c0:c1]
                    tv = t[:, c0:c1]
                    if df == 0:
                        nc.scalar.activation(
                            out=sv, in_=x[:, base + c0:base + c1],
                            func=mybir.ActivationFunctionType.Square,
                        )
                    else:
                        nc.vector.tensor_tensor(
                            out=sv,
                            in0=x[:, base + c0:base + c1],
                            in1=x[:, base + c0 + df:base + c1 + df],
                            op=MULT,
                        )
                    nc.scalar.activation(
                        out=ev, in_=sv, func=mybir.ActivationFunctionType.Exp
                    )
                    nc.vector.tensor_tensor(out=tv, in0=ev, in1=sv, op=MULT)
                return s, e, t, df

            def emit_preadd(tiles):
                s, e, t, df = tiles
                # collapse the two views into one via a DVE pre-add
                ue = upool.tile([P, LA], BF16, tag="ue")
                ut = upool.tile([P, LA], BF16, tag="ut")
                nc.vector.tensor_tensor(
                    out=ue[:], in0=e[:, T0:T0 + LA],
                    in1=e[:, T0 - df:T0 - df + LA], op=ADD)
                nc.vector.tensor_tensor(
                    out=ut[:], in0=t[:, T0:T0 + LA],
                    in1=t[:, T0 - df:T0 - df + LA], op=ADD)
                return ue, ut

            def emit_mms(k, tiles, pe, pt, first, chunks=range(N_CHUNKS),
                         qs="et", pre=None):
                s, e, t, df = tiles
                stop = k == len(CANON) - 1
                if pre is not None:
                    ue, ut = pre
                    views = [(ue, ut, 0)]
                else:
                    views = [(e, t, T0)]
                    if df != 0:
                        views.append((e, t, T0 - df))
                    if k == 0:
                        # the T0-df view's first chunks only need the first
                        # input cut, so emit that view first at kernel start
                        views.reverse()
                for esrc, tsrc, to in views:
                    for ci in chunks:
                        if "e" in qs:
                            pev = pe[ci // 2][:, (ci % 2) * MM_CHUNK:
                                              (ci % 2 + 1) * MM_CHUNK]
                            mve = view2d(esrc[:], to + ci * RPC * WP,
                                         RPC, W, WP)
                            nc.tensor.matmul(pev, ident[:], mve,
                                             start=first[ci], stop=stop)
                        if "t" in qs:
                            ptv = pt[ci // 2][:, (ci % 2) * MM_CHUNK:
                                              (ci % 2 + 1) * MM_CHUNK]
                            mvt = view2d(tsrc[:], to + ci * RPC * WP,
                                         RPC, W, WP)
                            nc.tensor.matmul(ptv, ident[:], mvt,
                                             start=first[ci], stop=stop)
                        if qs == "et":
                            first[ci] = False

            def emit_one(k, tiles, pe, pt, first, chunks=range(N_CHUNKS)):
                pre = emit_preadd(tiles) if CANON[k] in PREADD else None
                emit_mms(k, tiles, pe, pt, first, chunks, pre=pre)

            def emit_final_dr(b, h, pe):
                base = b * NX
                xc = view2d(x[:], base + T0 + h * (BR // 2) * WP,
                            BR // 2, W, WP)
                den = fin.tile([P, HALF], F32, tag="den%d" % h)
                r = fin.tile([P, HALF], F32, tag="r%d" % h)
                nc.vector.tensor_tensor(out=den[:], in0=pe[h][:], in1=xc,
                                        op=MULT)
                nc.vector.reciprocal_approx_fast(out=r[:], in_=den[:])
                return r

            def emit_final_out(b, h, pt, r):
                out = fin.tile([P, HALF], BF16, tag="out%d" % h)
                nc.vector.tensor_tensor(out=out[:], in0=pt[h][:], in1=r[:],
                                        op=MULT)
                nc.sync.dma_start(
                    out=y_out[:, b * LC + h * HALF:b * LC + (h + 1) * HALF],
                    in_=out[:])

            def emit_batch_tail(b, pe, pt, first, last=False):
                # all e-matmuls of the last N_TAIL tiles before their
                # t-matmuls, in chunk-pair order: closes each PSUM half
                # early so den/recip/out run under the trailing matmuls
                ks = list(range(len(CANON) - N_TAIL, len(CANON)))
                tls = [emit_tile(b, k) for k in ks]
                rs = [None, None]
                for cp in ((0, 1), (2, 3)):
                    for k, tl in zip(ks, tls):
                        emit_mms(k, tl, pe, pt, first, chunks=cp, qs="e")
                    rs[cp[0] // 2] = emit_final_dr(b, cp[0] // 2, pe)
                for cp in ((0, 1), (2, 3)):
                    for k, tl in zip(ks, tls):
                        emit_mms(k, tl, pe, pt, first, chunks=cp, qs="t")
                    if last:
                        emit_final_out(b, cp[0] // 2, pt, rs[cp[0] // 2])
                return rs

            pt_prev = rs_prev = None
            n_mid = len(CANON) - N_TAIL
            for b in range(N_BATCH):
                pe = [ps.tile([P, HALF], F32, tag="pe%d" % i,
                              name="pe%d" % i) for i in (0, 1)]
                pt = [ps.tile([P, HALF], F32, tag="pt%d" % i,
                              name="pt%d" % i) for i in (0, 1)]
                first = [True] * N_CHUNKS
                if b == 0:
                    for k in range(n_mid):
                        emit_one(k, emit_tile(b, k), pe, pt, first)
                else:
                    # den/recip for the previous batch already ran under its
                    # tail; only out+DMA (which release the pt banks) remain,
                    # interleaved with this batch's first tile computes
                    t0_tiles = emit_tile(b, 0)
                    emit_final_out(b - 1, 0, pt_prev, rs_prev[0])
                    emit_one(0, t0_tiles, pe, pt, first, chunks=(0, 1))
                    t1_tiles = emit_tile(b, 1)
                    emit_final_out(b - 1, 1, pt_prev, rs_prev[1])
                    emit_one(0, t0_tiles, pe, pt, first, chunks=(2, 3))
                    emit_one(1, t1_tiles, pe, pt, first)
                    for k in range(2, n_mid):
                        emit_one(k, emit_tile(b, k), pe, pt, first)
                rs_prev = emit_batch_tail(b, pe, pt, first,
                                          last=(b == N_BATCH - 1))
                pt_prev = pt
    nc.compile()
    return nc


_NC_CACHE = {}


def _get_nc():
    if "nc" not in _NC_CACHE:
        _NC_CACHE["nc"] = build_nc()
    return _NC_CACHE["nc"]


def make_slabs(imgs):
    """[32,128,128] fp32 (one core) -> [128, 2*3808] bf16 slab layout."""
    xb = imgs.astype(NP_BF16)
    xp = np.pad(xb, ((0, 0), (PADV, PADV), (PADH, PADH)))
    rows = (np.arange(RB_N) * BR)[:, None] + np.arange(SLAB)
    out = np.empty((P, N_BATCH, NX), NP_BF16)
    for b in range(N_BATCH):
        part = xp[b * IMG_PER_BATCH:(b + 1) * IMG_PER_BATCH]  # [16,140,136]
        sl = part[:, rows, :]              # [16, 8, 28, 136]
        sl = sl.transpose(1, 0, 2, 3)      # [8, 16, 28, 136] p = rb*16+img
        out[:, b, :] = sl.reshape(P, NX)
    return np.ascontiguousarray(out.reshape(P, N_BATCH * NX))


def unslab_out(y):
    """[128, 2*2048] bf16 -> [32, 128, 128] fp32."""
    y = y.astype(np.float32)
    res = np.empty((IMG_PER_CORE, H, W), np.float32)
    for b in range(N_BATCH):
        yb = y[:, b * LC:(b + 1) * LC].reshape(RB_N, IMG_PER_BATCH, BR, W)
        res[b * IMG_PER_BATCH:(b + 1) * IMG_PER_BATCH] = (
            yb.transpose(1, 0, 2, 3).reshape(IMG_PER_BATCH, H, W)
        )
    return res


def run(x, **spmd_kwargs):
    nc = _get_nc()
    imgs = np.ascontiguousarray(np.asarray(x).reshape(N_IMG_TOTAL, H, W))
    imgs = imgs.astype(np.float32, copy=False)
    ident = np.eye(P, dtype=NP_BF16)
    in_maps = [
        {"x": make_slabs(imgs[i * IMG_PER_CORE:(i + 1) * IMG_PER_CORE]),
         "ident": ident}
        for i in range(N_CORES)
    ]
    res = run_bass_kernel_spmd(nc, in_maps, core_ids=list(range(N_CORES)),
                               **spmd_kwargs)
    out = np.concatenate(
        [unslab_out(res.results[i]["y"]) for i in range(N_CORES)],
        axis=0,
    )
    return out.reshape(B, C, H, W).astype(np.float32, copy=False), res


def kernel(x):
    out, _ = run(x)
    return out
